# revision 15
# baseline (speedup 1.0000x reference)
"""C51 categorical-DQN histogram projection on Trainium2, 8-core data-parallel.

Direct-shift reformulation (no DRAM prefix table, no indirect DMA):
  m[b,j] = sum_k au_k[j] * pS[j+k], k in {-1,0,1}
  pS[n] = p_eff[n - s] (zero outside), s = floor((alpha+0.48)/gamma),
  au_k[j] = relu(1 - |rho + gamma*k - 0.01 j|), rho = alpha - gamma*s in
  [-0.48, 0.51].  The per-row integer shift s is applied in-SBUF by a
  two-level (base-5) cascade of copy_predicated selects over a zero-padded
  per-row copy of p (bf16).  The three tap weights are computed in ONE
  activation pair over a 3-section table (Y3 = rho - 0.01 j + gamma k).
  Edge bins are overwritten exactly (both windows in one op via two-window
  access patterns):
    m[0]  = sum_{a<16}  p[a] * clip(1 - alpha - g a, 0, 1) * mask
    m[50] = sum_{a>=35} p[a] * clip(alpha + g a - 49, 0, 1) * mask
  mask=0 rows use p_eff = onehot(25), alpha = q - 24.75 (q = clip(2.5r+25)).
Row mapping: row = p*1024 + t*G + g (contiguous per-partition scalars).
Note: f32->i32 tensor_copy ROUNDS on hardware (trunc in CoreSim); the
shift digits are derived with pure integer ops so both agree, and the
floor() offsets tolerate +-1 (the rho window [-0.48, 0.98] absorbs it).
"""
import sys
sys.path.insert(0, "/opt/trn_rl_repo")
import numpy as np
from concourse import bass, bacc, mybir, tile
from concourse.bass_utils import run_bass_kernel_spmd

F32 = mybir.dt.float32
BF16 = mybir.dt.bfloat16
I32 = mybir.dt.int32
I16 = mybir.dt.int16
OP = mybir.AluOpType
AF = mybir.ActivationFunctionType

P = 128
A = 51
B_TOTAL = 1048576
N_CORES = 8
BC = B_TOTAL // N_CORES
G = 64
TILE = P * G
T = BC // TILE          # 16 tiles/core
TG = BC // P            # 1024 scalars per partition
GAMMA = 0.99
PADL = 17
WPB = 85                # padded p width per group: zeros outside [17,68)
WT1 = 58                # coarse-select output width
WT1S = 60               # T1 storage stride
WPS = 53                # fine-select output width (pS[n], n in [-1,51])
WPSS = 54               # pS storage stride
EW = 16                 # edge-window atoms per side
W3 = 3 * A              # 3-section tap table width


def _host_consts():
    import ml_dtypes
    j = np.arange(A, dtype=np.float32)
    j3 = np.concatenate([-0.01 * j + GAMMA * k for k in (-1, 0, 1)])
    j3 = j3[None, :].repeat(P, 0).astype(ml_dtypes.bfloat16)
    tabL = -GAMMA * np.arange(EW, dtype=np.float32)
    tabR = GAMMA * (35.0 + np.arange(EW, dtype=np.float32)) - 48.0
    tabLR = np.concatenate([tabL, tabR])[None, :].repeat(P, 0).astype(
        ml_dtypes.bfloat16)
    return j3, tabLR


def _build_nc(Bc, repeat=1):
    from contextlib import nullcontext
    FA = G * A
    T_ = Bc // TILE
    TG_ = Bc // P
    nc = bacc.Bacc("TRN2", target_bir_lowering=False, debug=False)
    pr = nc.dram_tensor("pdist", [Bc, A], F32, kind="ExternalInput")
    rr = nc.dram_tensor("reward", [Bc], F32, kind="ExternalInput")
    mm = nc.dram_tensor("mask", [Bc], I32, kind="ExternalInput")
    j3_c = nc.dram_tensor("j3", [P, W3], BF16, kind="ExternalInput")
    tab_c = nc.dram_tensor("tabLR", [P, 2 * EW], BF16, kind="ExternalInput")
    mo = nc.dram_tensor("mout", [Bc, A], F32, kind="ExternalOutput")

    prf = pr[:, :].rearrange("b a -> (b a)")
    mof = mo[:, :].rearrange("b a -> (b a)")

    with tile.TileContext(nc) as tc:
      with (tc.For_i(0, repeat) if repeat > 1 else nullcontext()):
        with tc.tile_pool(name="const", bufs=1) as cpool:
            j3 = cpool.tile([P, W3], BF16)
            nc.sync.dma_start(out=j3[:], in_=j3_c[:, :])
            tabLR = cpool.tile([P, 2 * EW], BF16)
            nc.sync.dma_start(out=tabLR[:], in_=tab_c[:, :])
            bone = cpool.tile([P, 1], F32, tag="bone")
            nc.vector.memset(bone[:], 1.0)

            # ---- prepass: all per-row scalars for the whole core ----
            mfv = cpool.tile([P, TG_], F32, tag="mfv")
            rho = cpool.tile([P, TG_], F32, tag="rho")
            pnq = cpool.tile([P, TG_ * 2], BF16, tag="pnq")
            nm16 = cpool.tile([P, TG_], BF16, tag="nm16")
            rho16 = cpool.tile([P, TG_], BF16, tag="rho16")
            mcs = {c: cpool.tile([P, TG_], I16, tag=f"mc{c}", name=f"mc{c}")
                   for c in range(1, 6)}
            mfs = {f: cpool.tile([P, TG_], I16, tag=f"mf{f}", name=f"mf{f}")
                   for f in range(1, 5)}
            w0 = cpool.tile([P, TG_], F32, tag="w0")
            w1 = cpool.tile([P, TG_], F32, tag="w1")
            w2 = cpool.tile([P, TG_], F32, tag="w2")
            si = cpool.tile([P, TG_], I32, tag="si")
            ci = cpool.tile([P, TG_], I32, tag="ci")
            fi = cpool.tile([P, TG_], I32, tag="fi")

            nc.sync.dma_start(out=w0[:], in_=bass.AP(rr[:].tensor, 0,
                                                     [[TG_, P], [1, TG_]]))
            nc.sync.dma_start(out=si[:], in_=bass.AP(mm[:].tensor, 0,
                                                     [[TG_, P], [1, TG_]]))
            nc.gpsimd.tensor_copy(out=mfv[:], in_=si[:])
            nc.gpsimd.tensor_scalar(out=w2[:], in0=mfv[:], scalar1=-1.0,
                                    scalar2=1.0, op0=OP.mult, op1=OP.add)
            nc.gpsimd.tensor_copy(out=nm16[:], in_=w2[:])
            # w2 = aq = clip(2.5r+25, 0, 50) - 24.75
            nc.gpsimd.tensor_scalar(out=w1[:], in0=w0[:], scalar1=2.5,
                                    scalar2=25.0, op0=OP.mult, op1=OP.add)
            nc.gpsimd.tensor_scalar(out=w1[:], in0=w1[:], scalar1=0.0,
                                    scalar2=50.0, op0=OP.max, op1=OP.min)
            nc.gpsimd.tensor_scalar(out=w2[:], in0=w1[:], scalar1=-24.75,
                                    scalar2=None, op0=OP.add)
            # w1 = al = mf*(a1 - aq) + aq
            nc.gpsimd.tensor_scalar(out=w1[:], in0=w0[:], scalar1=2.5,
                                    scalar2=0.25, op0=OP.mult, op1=OP.add)
            nc.gpsimd.tensor_tensor(out=w1[:], in0=w1[:], in1=w2[:],
                                    op=OP.subtract)
            nc.gpsimd.tensor_tensor(out=w1[:], in0=w1[:], in1=mfv[:],
                                    op=OP.mult)
            nc.gpsimd.tensor_tensor(out=w1[:], in0=w1[:], in1=w2[:],
                                    op=OP.add)
            # pnq interleaved (oma, -oma): oma = 1 - al
            nc.gpsimd.tensor_scalar(out=w0[:], in0=w1[:], scalar1=-1.0,
                                    scalar2=1.0, op0=OP.mult, op1=OP.add)
            pq = pnq[:]
            nc.gpsimd.tensor_copy(
                out=bass.AP(pq.tensor, pq.offset, [pq.ap[0], [2, TG_]]),
                in_=w0[:])
            nc.gpsimd.tensor_scalar(out=w0[:], in0=w0[:], scalar1=-1.0,
                                    scalar2=None, op0=OP.mult)
            nc.gpsimd.tensor_copy(
                out=bass.AP(pq.tensor, pq.offset + 1, [pq.ap[0], [2, TG_]]),
                in_=w0[:])
            # si = round((al+0.48)/g + 16 - 0.5); rho = al - 0.99*si + 15.84
            nc.gpsimd.tensor_scalar(out=w0[:], in0=w1[:],
                                    scalar1=1.0101010101010102,
                                    scalar2=15.984848484848484,
                                    op0=OP.mult, op1=OP.add)
            nc.gpsimd.tensor_copy(out=si[:], in_=w0[:])
            nc.gpsimd.tensor_copy(out=w2[:], in_=si[:])
            nc.gpsimd.tensor_scalar(out=w2[:], in0=w2[:], scalar1=-GAMMA,
                                    scalar2=15.84, op0=OP.mult, op1=OP.add)
            nc.gpsimd.tensor_tensor(out=rho[:], in0=w2[:], in1=w1[:],
                                    op=OP.add)
            nc.gpsimd.tensor_copy(out=rho16[:], in_=rho[:])
            # si := delta = 32 - si in [2,29]; coarse by is_ge cascade,
            # c = sum(masks), f = delta - 5c in [0,4] (pure integer).
            nc.gpsimd.tensor_scalar(out=si[:], in0=si[:], scalar1=-1,
                                    scalar2=32, op0=OP.mult, op1=OP.add)
            for c in range(1, 6):
                nc.gpsimd.tensor_scalar(out=fi[:], in0=si[:],
                                        scalar1=5 * c, scalar2=None,
                                        op0=OP.is_ge)
                nc.gpsimd.tensor_copy(out=mcs[c][:], in_=fi[:])
                if c == 1:
                    nc.gpsimd.tensor_copy(out=ci[:], in_=fi[:])
                else:
                    nc.gpsimd.tensor_tensor(out=ci[:], in0=ci[:],
                                            in1=fi[:], op=OP.add)
            nc.gpsimd.tensor_scalar(out=fi[:], in0=ci[:], scalar1=-5,
                                    scalar2=None, op0=OP.mult)
            nc.gpsimd.tensor_tensor(out=fi[:], in0=fi[:], in1=si[:],
                                    op=OP.add)
            for f in range(1, 5):
                nc.gpsimd.tensor_scalar(out=ci[:], in0=fi[:], scalar1=f,
                                        scalar2=None, op0=OP.is_equal)
                nc.gpsimd.tensor_copy(out=mfs[f][:], in_=ci[:])

            def gview(tl, t, w):
                h = tl[:]
                return bass.AP(h.tensor, h.offset + t * G,
                               [h.ap[0], [1, G], [0, w]])

            with tc.tile_pool(name="io", bufs=2) as iop, \
                 tc.tile_pool(name="wk2", bufs=2) as wk2, \
                 tc.tile_pool(name="wk1", bufs=1) as wk1:
                for t in range(T_):
                    pt = iop.tile([P, FA], F32, tag="pt")
                    nc.sync.dma_start(
                        out=pt[:],
                        in_=bass.AP(prf.tensor, t * G * A,
                                    [[TG_ * A, P], [A, G], [1, A]]))

                    # padded p_eff (bf16), single two-window pad memset
                    PB = wk2.tile([P, G * WPB], BF16, tag="PB")

                    def pbv(off, w):
                        h = PB[:]
                        return bass.AP(h.tensor, h.offset + off,
                                       [h.ap[0], [WPB, G], [1, w]])

                    pbh = PB[:]
                    nc.gpsimd.memset(
                        bass.AP(pbh.tensor, pbh.offset,
                                [pbh.ap[0], [WPB, G], [PADL + A, 2],
                                 [1, PADL]]), 0.0)
                    nc.gpsimd.tensor_tensor(
                        out=pbv(PADL, A), in0=pt[:], in1=gview(mfv, t, A),
                        op=OP.mult)
                    pbcol = bass.AP(pbh.tensor, pbh.offset + PADL + 25,
                                    [pbh.ap[0], [WPB, G]])
                    nc.gpsimd.tensor_tensor(
                        out=pbcol, in0=pbcol,
                        in1=bass.AP(nm16[:].tensor, nm16[:].offset + t * G,
                                    [nm16[:].ap[0], [1, G]]), op=OP.add)

                    # tap weights: au3 = relu(1 - |rho - 0.01 j + g k|),
                    # all 3 sections in one op pair
                    au3 = wk1.tile([P, G * W3], BF16, tag="au3")
                    nc.vector.tensor_tensor(
                        out=au3[:], in0=gview(rho16, t, W3),
                        in1=bass.AP(j3[:].tensor, j3[:].offset,
                                    [j3[:].ap[0], [0, G], [1, W3]]),
                        op=OP.add)
                    nc.scalar.activation(out=au3[:], in_=au3[:], func=AF.Abs,
                                         bias=0.0, scale=1.0)
                    nc.scalar.activation(out=au3[:], in_=au3[:], func=AF.Relu,
                                         bias=bone[:], scale=-1.0)

                    def auv(k):
                        h = au3[:]
                        return bass.AP(h.tensor, h.offset + (k + 1) * A,
                                       [h.ap[0], [W3, G], [1, A]])

                    # coarse select: T1[x] = PB[x + 5c]
                    T1 = wk1.tile([P, G * WT1S], BF16, tag="T1")

                    def t1v(off, w):
                        h = T1[:]
                        return bass.AP(h.tensor, h.offset + off,
                                       [h.ap[0], [WT1S, G], [1, w]])

                    nc.vector.tensor_copy(out=t1v(0, WT1), in_=pbv(0, WT1))
                    for c in range(1, 6):
                        nc.vector.copy_predicated(
                            out=t1v(0, WT1), mask=gview(mcs[c], t, WT1),
                            data=pbv(5 * c, WT1))
                    # fine select: pS[x] = T1[x + f]
                    pS = wk1.tile([P, G * WPSS], BF16, tag="pS")

                    def psv(off, w):
                        h = pS[:]
                        return bass.AP(h.tensor, h.offset + off,
                                       [h.ap[0], [WPSS, G], [1, w]])

                    nc.vector.tensor_copy(out=psv(0, WPS), in_=t1v(0, WPS))
                    for f in range(1, 5):
                        nc.vector.copy_predicated(
                            out=psv(0, WPS), mask=gview(mfs[f], t, WPS),
                            data=t1v(f, WPS))

                    # taps (bf16 accumulate, final add widens to f32)
                    mt = iop.tile([P, FA], F32, tag="mt")
                    tm1 = wk1.tile([P, FA], BF16, tag="tm1")
                    tm2 = wk1.tile([P, FA], BF16, tag="tm2")
                    nc.vector.tensor_tensor(out=tm1[:], in0=auv(-1),
                                            in1=psv(0, A), op=OP.mult)
                    nc.vector.tensor_tensor(out=tm2[:], in0=auv(0),
                                            in1=psv(1, A), op=OP.mult)
                    nc.vector.tensor_tensor(out=tm1[:], in0=tm1[:],
                                            in1=tm2[:], op=OP.add)
                    nc.vector.tensor_tensor(out=tm2[:], in0=auv(1),
                                            in1=psv(2, A), op=OP.mult)
                    nc.vector.tensor_tensor(out=mt[:], in0=tm1[:],
                                            in1=tm2[:], op=OP.add)

                    # edges: both windows in one op chain; PB already
                    # carries p*mask (one-hot col 42 is outside windows)
                    wc = wk2.tile([P, G * 2 * EW], BF16, tag="wc")

                    def wcv(dims):
                        h = wc[:]
                        return bass.AP(h.tensor, h.offset, [h.ap[0]] + dims)

                    nc.gpsimd.tensor_tensor(
                        out=wcv([[2 * EW, G], [1, 2 * EW]]),
                        in0=bass.AP(tabLR[:].tensor, tabLR[:].offset,
                                    [tabLR[:].ap[0], [0, G], [1, 2 * EW]]),
                        in1=bass.AP(pnq[:].tensor, pnq[:].offset + t * 2 * G,
                                    [pnq[:].ap[0], [2, G], [1, 2], [0, EW]]),
                        op=OP.add)
                    nc.gpsimd.tensor_scalar(out=wc[:], in0=wc[:], scalar1=0.0,
                                            scalar2=1.0, op0=OP.max,
                                            op1=OP.min)
                    nc.gpsimd.tensor_tensor(
                        out=wcv([[2 * EW, G], [1, 2 * EW]]),
                        in0=wcv([[2 * EW, G], [1, 2 * EW]]),
                        in1=bass.AP(pbh.tensor, pbh.offset + PADL,
                                    [pbh.ap[0], [WPB, G], [35, 2], [1, EW]]),
                        op=OP.mult)
                    m05 = wk2.tile([P, G * 2], F32, tag="m05")
                    nc.vector.tensor_reduce(
                        out=bass.AP(m05[:].tensor, m05[:].offset,
                                    [m05[:].ap[0], [2, G], [1, 2]]),
                        in_=wcv([[2 * EW, G], [EW, 2], [1, EW]]),
                        axis=mybir.AxisListType.X, op=OP.add)
                    mth = mt[:]
                    nc.gpsimd.tensor_copy(
                        out=bass.AP(mth.tensor, mth.offset,
                                    [mth.ap[0], [A, G], [50, 2]]),
                        in_=bass.AP(m05[:].tensor, m05[:].offset,
                                    [m05[:].ap[0], [2, G], [1, 2]]))

                    nc.sync.dma_start(
                        out=bass.AP(mof.tensor, t * G * A,
                                    [[TG_ * A, P], [A, G], [1, A]]),
                        in_=mt[:])
    nc.compile()
    return nc


_NC_CACHE = {}


def kernel(batch_reward, max_next_dist, supports, non_final_mask):
    assert max_next_dist.shape == (B_TOTAL, A)
    if "nc" not in _NC_CACHE:
        _NC_CACHE["nc"] = _build_nc(BC)
    nc = _NC_CACHE["nc"]
    j3, tabLR = _host_consts()
    in_maps = []
    for c in range(N_CORES):
        s = slice(c * BC, (c + 1) * BC)
        in_maps.append({
            "pdist": np.ascontiguousarray(max_next_dist[s]).astype(np.float32),
            "reward": np.ascontiguousarray(batch_reward[s]).astype(np.float32),
            "mask": np.ascontiguousarray(non_final_mask[s]).astype(np.int32),
            "j3": j3, "tabLR": tabLR,
        })
    res = run_bass_kernel_spmd(nc, in_maps, core_ids=list(range(N_CORES)))
    return np.concatenate([res.results[c]["mout"] for c in range(N_CORES)],
                          axis=0)
